# revision 6
# baseline (speedup 1.0000x reference)
"""PointNet MLP (3 x conv1x1+BN+ReLU, final valid-mask) on 8 TRN2 cores.

Sharding: compacted-column parallel. The valid mask keeps ~70% of the
4096*128 = 524288 point-neighbor columns; masked columns are exactly 0 in
the reference output. Host gathers the valid columns, splits them evenly
across 8 cores, device computes only those, host scatters into zeros.

Numerics: plain fp16 weights/activations with f32 PSUM accumulation and
fp16 output (end-to-end rel err ~7e-4 vs the 2e-2 gate).

Device per-core loop, software-pipelined 3 stages deep so the PE never
waits on the activation engines.  Iteration = 2048 stream columns, with
the first 1024 columns packed on PSUM partitions 0-63 and the second
1024 on partitions 64-127 for layers 1/2 (64 channels each):
 - stage A(t): mm1 = one K=6 M=1024 matmul (lhsT [6,128] holds W1'
   twice), act1 = ACT Relu(ps1 + b1cat) -> fp16.
 - stage B(t-1): mm2 = one K=128 block-diag matmul, act2 = ACT
   Relu(ps2 + b2cat) -> fp16.
 - stage C(t-2): mm3 = two K=64 M=1024 matmuls (cols 0:1024 from hi2
   partitions 0:64, cols 1024:2048 from partitions 64:128), act3 = DVE
   max(ps3 + b3, 0) -> fp16 [128,2048], DMA out.
BN is folded into conv weights/bias on host (f64); host casts the fp16
result to f32 and scatters into the zero-initialized full output.
"""

import numpy as np

try:
    import concourse.bass as bass
except ImportError:
    import sys

    sys.path.insert(0, "/opt/trn_rl_repo")
    import concourse.bass as bass

import concourse.bacc as bacc

import concourse.mybir as mybir
from concourse import tile
from concourse.bass_utils import run_bass_kernel_spmd

F32 = mybir.dt.float32
F16 = mybir.dt.float16

N_CORES = 8
NPOINT, KNN = 4096, 128
NCOLS = NPOINT * KNN
HALF = 1024
ITER_COLS = 2 * HALF  # 2048 stream columns per iteration
EPS = 1e-5

# Moving-dim size per matmul (the ISA caps the moving dim at 512).
MM_M = 512

_NC_CACHE = {}


def _build_nc(iters):
    nc = bacc.Bacc("TRN2", target_bir_lowering=False)
    xp_d = nc.declare_dram_parameter("xp", [6, iters * HALF], F16, isOutput=False)
    w1_d = nc.declare_dram_parameter("lhsT1", [6, 128], F16, isOutput=False)
    w2_d = nc.declare_dram_parameter("lhsT2", [128, 128], F16, isOutput=False)
    w3_d = nc.declare_dram_parameter("lhsT3", [128, 128], F16, isOutput=False)
    bias_d = nc.declare_dram_parameter("biases", [128, 3], F32, isOutput=False)
    out_d = nc.declare_dram_parameter("out", [128, iters * ITER_COLS], F16,
                                      isOutput=True)

    add = mybir.AluOpType.add
    vmax = mybir.AluOpType.max
    relu_fn = mybir.ActivationFunctionType.Relu

    with tile.TileContext(nc) as tc:
        with (
            tc.tile_pool(name="const", bufs=1) as cpool,
            tc.tile_pool(name="xpool", bufs=1) as xpool,
            tc.tile_pool(name="ypool", bufs=2) as ypool,
            tc.tile_pool(name="opool", bufs=3) as opool,
            tc.tile_pool(name="pspool", bufs=1, space="PSUM") as pspool,
        ):
            w1_sb = cpool.tile([6, 128], F16, tag="w1")
            w2_sb = cpool.tile([128, 128], F16, tag="w2")
            w3_sb = cpool.tile([128, 128], F16, tag="w3")
            bias_sb = cpool.tile([128, 3], F32, tag="bias")
            nc.sync.dma_start(w1_sb[:, :], w1_d[:, :])
            nc.sync.dma_start(w2_sb[:, :], w2_d[:, :])
            nc.sync.dma_start(w3_sb[:, :], w3_d[:, :])
            nc.sync.dma_start(bias_sb[:, :], bias_d[:, :])
            b1_ap = bias_sb[:, 0:1]
            b2_ap = bias_sb[:, 1:2]
            b3_ap = bias_sb[:, 2:3]

            xcols = iters * HALF
            x_sb = xpool.tile([6, xcols], F16, tag="x")
            nch = 4
            chunk = -(-xcols // nch)
            for c in range(nch):
                lo = c * chunk
                hi = min(xcols, lo + chunk)
                if hi > lo:
                    nc.sync.dma_start(x_sb[:, lo:hi], xp_d[:, lo:hi])

            # Per-stage tiles, indexed by pipeline step.
            tiles = {}

            def mk(t):
                tiles[t] = dict(
                    ps1=pspool.tile([128, HALF], F32, tag="ps1", name=f"ps1_{t}"),
                    ps2=pspool.tile([128, HALF], F32, tag="ps2", name=f"ps2_{t}"),
                    ps3=pspool.tile([128, ITER_COLS], F32, tag="ps3",
                                    name=f"ps3_{t}"),
                    hi1=ypool.tile([128, HALF], F16, tag="hi1", name=f"hi1_{t}"),
                    hi2=ypool.tile([128, HALF], F16, tag="hi2", name=f"hi2_{t}"),
                    ob=opool.tile([128, ITER_COLS], F16, tag="ob", name=f"ob_{t}"),
                )

            for t in range(iters + 2):
                if t < iters:
                    mk(t)
                    c0 = t * HALF
                    d = tiles[t]
                    for m0 in range(0, HALF, MM_M):
                        nc.tensor.matmul(d["ps1"][:, m0 : m0 + MM_M],
                                         w1_sb[:, :],
                                         x_sb[:, c0 + m0 : c0 + m0 + MM_M])
                if t >= 1 and t - 1 < iters:
                    d = tiles[t - 1]
                    for m0 in range(0, HALF, MM_M):
                        nc.tensor.matmul(d["ps2"][:, m0 : m0 + MM_M],
                                         w2_sb[:, :],
                                         d["hi1"][:, m0 : m0 + MM_M])
                if t >= 2:
                    d = tiles[t - 2]
                    for m0 in range(0, HALF, MM_M):
                        nc.tensor.matmul(d["ps3"][:, m0 : m0 + MM_M],
                                         w3_sb[0:64, :],
                                         d["hi2"][0:64, m0 : m0 + MM_M])
                    for m0 in range(0, HALF, MM_M):
                        nc.tensor.matmul(d["ps3"][:, HALF + m0 : HALF + m0 + MM_M],
                                         w3_sb[64:128, :],
                                         d["hi2"][64:128, m0 : m0 + MM_M])

                if t < iters:
                    d = tiles[t]
                    nc.scalar.activation(d["hi1"][:, :], d["ps1"][:, :],
                                         relu_fn, bias=b1_ap)
                if t >= 1 and t - 1 < iters:
                    d = tiles[t - 1]
                    nc.scalar.activation(d["hi2"][:, :], d["ps2"][:, :],
                                         relu_fn, bias=b2_ap)
                if t >= 2:
                    d = tiles[t - 2]
                    nc.vector.tensor_scalar(d["ob"][:, :], d["ps3"][:, :],
                                            b3_ap, 0.0, add, vmax)
                    o0 = (t - 2) * ITER_COLS
                    nc.sync.dma_start(out_d[:, o0 : o0 + ITER_COLS],
                                      d["ob"][:, :])
                    del tiles[t - 2]

    nc.compile()
    return nc


def _get_nc(iters):
    if iters not in _NC_CACHE:
        _NC_CACHE[iters] = _build_nc(iters)
    return _NC_CACHE[iters]


def _fold_bn(W, b, gamma, beta, mean, var):
    inv = gamma.astype(np.float64) / np.sqrt(var.astype(np.float64) + EPS)
    Wp = (W.astype(np.float64) * inv[:, None]).astype(np.float32)
    bp = ((b.astype(np.float64) - mean.astype(np.float64)) * inv
          + beta.astype(np.float64)).astype(np.float32)
    return Wp, bp


def _prepare(inputs):
    gp = np.asarray(inputs["grouped_pc"], dtype=np.float32)
    valid = np.asarray(inputs["valid"], dtype=np.float32)

    Wp1, bp1 = _fold_bn(*(np.asarray(inputs[k], dtype=np.float32)
                          for k in ("W1", "b1", "gamma1", "beta1", "mean1", "var1")))
    Wp2, bp2 = _fold_bn(*(np.asarray(inputs[k], dtype=np.float32)
                          for k in ("W2", "b2", "gamma2", "beta2", "mean2", "var2")))
    Wp3, bp3 = _fold_bn(*(np.asarray(inputs[k], dtype=np.float32)
                          for k in ("W3", "b3", "gamma3", "beta3", "mean3", "var3")))

    lhsT1 = np.zeros((6, 128), np.float16)
    lhsT1[0:3, 0:64] = Wp1.T
    lhsT1[3:6, 64:128] = Wp1.T

    lhsT2 = np.zeros((128, 128), np.float16)
    lhsT2[0:64, 0:64] = Wp2.T
    lhsT2[64:128, 64:128] = Wp2.T

    lhsT3 = np.zeros((128, 128), np.float16)
    lhsT3[0:64, :] = Wp3.T
    lhsT3[64:128, :] = Wp3.T

    biases = np.zeros((128, 3), np.float32)
    biases[:, 0] = np.concatenate([bp1, bp1])
    biases[:, 1] = np.concatenate([bp2, bp2])
    biases[:, 2] = bp3

    x = gp[0].reshape(3, NCOLS)
    vidx = np.flatnonzero(valid.reshape(NCOLS) > 0.5)
    V = len(vidx)
    Vc = -(-V // N_CORES)
    iters = max(1, -(-Vc // ITER_COLS))
    cap = iters * ITER_COLS

    xv = x[:, vidx].astype(np.float16)

    in_maps = []
    for c in range(N_CORES):
        lo_i = c * Vc
        hi_i = min((c + 1) * Vc, V)
        n = max(0, hi_i - lo_i)
        xa = np.zeros((3, cap), np.float16)
        if n:
            xa[:, :n] = xv[:, lo_i:hi_i]
        xr = xa.reshape(3, iters, 2, HALF)
        xp = np.empty((6, iters, HALF), np.float16)
        xp[0:3] = xr[:, :, 0, :]
        xp[3:6] = xr[:, :, 1, :]
        in_maps.append(
            {
                "xp": np.ascontiguousarray(xp.reshape(6, iters * HALF)),
                "lhsT1": lhsT1,
                "lhsT2": lhsT2,
                "lhsT3": lhsT3,
                "biases": biases,
            }
        )
    return in_maps, vidx, V, Vc, iters


def _gather(results, vidx, V, Vc):
    stream = np.empty((128, V), np.float32)
    for c in range(N_CORES):
        lo_i = c * Vc
        hi_i = min((c + 1) * Vc, V)
        if hi_i <= lo_i:
            break
        stream[:, lo_i:hi_i] = results[c]["out"][:, : hi_i - lo_i]
    full = np.zeros((128, NCOLS), np.float32)
    full[:, vidx] = stream
    return full.reshape(128, NPOINT, KNN)[None]


def run_traced(trace=False, **inputs):
    in_maps, vidx, V, Vc, iters = _prepare(inputs)
    nc = _get_nc(iters)
    res = run_bass_kernel_spmd(nc, in_maps, list(range(N_CORES)), trace=trace)
    return _gather(res.results, vidx, V, Vc), res.exec_time_ns


def kernel(**inputs):
    out, _ = run_traced(trace=False, **inputs)
    return out


# revision 7
# speedup vs baseline: 1.4115x; 1.4115x over previous
"""PointNet MLP (3 x conv1x1+BN+ReLU, final valid-mask) on 8 TRN2 cores.

Sharding: compacted-column parallel. The valid mask keeps ~70% of the
4096*128 = 524288 point-neighbor columns; masked columns are exactly 0 in
the reference output. Host gathers the valid columns, splits them evenly
across 8 cores, device computes only those, host scatters into zeros.

Numerics: plain fp16 weights/activations with f32 PSUM accumulation and
fp16 output (end-to-end rel err ~7e-4 vs the 2e-2 gate).

Device schedule: iterations of 1024 stream columns (block-pair: first
512 on PSUM partitions 0-63, second 512 on 64-127 for the 64-channel
layers), software-pipelined two iterations per "superstep" so the PE
issues 8 back-to-back matmuls per superstep — long dense bursts keep
the tensor engine at its top p-state (interleaved single matmuls ran at
half clock).  All PSUM pools are double-buffered (8 banks exactly), so
every matmul's dependencies are one superstep old when it issues.

 - mm1: K=7 (xyz blockA, xyz blockB, ones) with b1 folded into lhsT.
   act1 = ACT Relu(ps1) -> fp16.
 - mm2: K=128 block-diag.  act2 = DVE max(ps2 + b2cat, 0) -> fp16.
 - mm3: 2x K=64 (lhsT3 rows 0:64 / 64:128).  act3 = Relu(ps3 + b3) ->
   fp16 [128,1024], alternating between ACT and DVE per iteration to
   balance engine load.  DMA out fp16; host casts to f32 and scatters.
BN folded into conv weights/bias on host (f64).
"""

import numpy as np

try:
    import concourse.bass as bass
except ImportError:
    import sys

    sys.path.insert(0, "/opt/trn_rl_repo")
    import concourse.bass as bass

import concourse.bacc as bacc

import concourse.mybir as mybir
from concourse import tile
from concourse.bass_utils import run_bass_kernel_spmd

F32 = mybir.dt.float32
F16 = mybir.dt.float16

N_CORES = 8
NPOINT, KNN = 4096, 128
NCOLS = NPOINT * KNN
M = 512
ITER_COLS = 2 * M  # 1024 stream columns per iteration
EPS = 1e-5

_NC_CACHE = {}


def _build_nc(iters):
    nc = bacc.Bacc("TRN2", target_bir_lowering=False)
    xp_d = nc.declare_dram_parameter("xp", [7, iters * M], F16, isOutput=False)
    w1_d = nc.declare_dram_parameter("lhsT1", [7, 128], F16, isOutput=False)
    w2_d = nc.declare_dram_parameter("lhsT2", [128, 128], F16, isOutput=False)
    w3_d = nc.declare_dram_parameter("lhsT3", [128, 128], F16, isOutput=False)
    bias_d = nc.declare_dram_parameter("biases", [128, 2], F32, isOutput=False)
    out_d = nc.declare_dram_parameter("out", [128, iters * ITER_COLS], F16,
                                      isOutput=True)

    add = mybir.AluOpType.add
    vmax = mybir.AluOpType.max
    relu_fn = mybir.ActivationFunctionType.Relu

    with tile.TileContext(nc) as tc:
        with (
            tc.tile_pool(name="const", bufs=1) as cpool,
            tc.tile_pool(name="xpool", bufs=1) as xpool,
            tc.tile_pool(name="ypool", bufs=4) as ypool,
            tc.tile_pool(name="opool", bufs=4) as opool,
            tc.tile_pool(name="pspool", bufs=2, space="PSUM") as pspool,
        ):
            w1_sb = cpool.tile([7, 128], F16, tag="w1")
            w2_sb = cpool.tile([128, 128], F16, tag="w2")
            w3_sb = cpool.tile([128, 128], F16, tag="w3")
            bias_sb = cpool.tile([128, 2], F32, tag="bias")
            nc.sync.dma_start(w1_sb[:, :], w1_d[:, :])
            nc.sync.dma_start(w2_sb[:, :], w2_d[:, :])
            nc.sync.dma_start(w3_sb[:, :], w3_d[:, :])
            nc.sync.dma_start(bias_sb[:, :], bias_d[:, :])
            b2_ap = bias_sb[:, 0:1]
            b3_ap = bias_sb[:, 1:2]

            xcols = iters * M
            x_sb = xpool.tile([7, xcols], F16, tag="x")
            nch = 4
            chunk = -(-xcols // nch)
            for c in range(nch):
                lo = c * chunk
                hi = min(xcols, lo + chunk)
                if hi > lo:
                    nc.sync.dma_start(x_sb[:, lo:hi], xp_d[:, lo:hi])

            tiles = {}

            def mk(t):
                tiles[t] = dict(
                    ps1=pspool.tile([128, M], F32, tag="ps1", name=f"ps1_{t}"),
                    ps2=pspool.tile([128, M], F32, tag="ps2", name=f"ps2_{t}"),
                    ps3=pspool.tile([128, ITER_COLS], F32, tag="ps3",
                                    name=f"ps3_{t}"),
                    hi1=ypool.tile([128, M], F16, tag="hi1", name=f"hi1_{t}"),
                    hi2=ypool.tile([128, M], F16, tag="hi2", name=f"hi2_{t}"),
                    ob=opool.tile([128, ITER_COLS], F16, tag="ob", name=f"ob_{t}"),
                )

            n_ss = -(-iters // 2) + 2
            for s in range(n_ss):
                a_iters = [i for i in (2 * s, 2 * s + 1) if i < iters]
                b_iters = [i for i in (2 * s - 2, 2 * s - 1) if 0 <= i < iters]
                c_iters = [i for i in (2 * s - 4, 2 * s - 3) if 0 <= i < iters]

                # PE burst: 8 back-to-back matmuls, all deps >= 1 superstep old.
                for i in a_iters:
                    mk(i)
                    d = tiles[i]
                    c0 = i * M
                    nc.tensor.matmul(d["ps1"][:, :], w1_sb[:, :],
                                     x_sb[:, c0 : c0 + M])
                for i in b_iters:
                    d = tiles[i]
                    nc.tensor.matmul(d["ps2"][:, :], w2_sb[:, :], d["hi1"][:, :])
                for i in c_iters:
                    d = tiles[i]
                    nc.tensor.matmul(d["ps3"][:, 0:M], w3_sb[0:64, :],
                                     d["hi2"][0:64, :])
                    nc.tensor.matmul(d["ps3"][:, M:ITER_COLS], w3_sb[64:128, :],
                                     d["hi2"][64:128, :])

                for i in a_iters:
                    d = tiles[i]
                    nc.scalar.activation(d["hi1"][:, :], d["ps1"][:, :], relu_fn)
                for i in b_iters:
                    d = tiles[i]
                    nc.vector.tensor_scalar(d["hi2"][:, :], d["ps2"][:, :],
                                            b2_ap, 0.0, add, vmax)
                for i in c_iters:
                    d = tiles[i]
                    if i % 2 == 0:
                        nc.scalar.activation(d["ob"][:, :], d["ps3"][:, :],
                                             relu_fn, bias=b3_ap)
                    else:
                        nc.vector.tensor_scalar(d["ob"][:, :], d["ps3"][:, :],
                                                b3_ap, 0.0, add, vmax)
                    o0 = i * ITER_COLS
                    nc.sync.dma_start(out_d[:, o0 : o0 + ITER_COLS],
                                      d["ob"][:, :])
                    del tiles[i]

    nc.compile()
    return nc


def _get_nc(iters):
    if iters not in _NC_CACHE:
        _NC_CACHE[iters] = _build_nc(iters)
    return _NC_CACHE[iters]


def _fold_bn(W, b, gamma, beta, mean, var):
    inv = gamma.astype(np.float64) / np.sqrt(var.astype(np.float64) + EPS)
    Wp = (W.astype(np.float64) * inv[:, None]).astype(np.float32)
    bp = ((b.astype(np.float64) - mean.astype(np.float64)) * inv
          + beta.astype(np.float64)).astype(np.float32)
    return Wp, bp


def _prepare(inputs):
    gp = np.asarray(inputs["grouped_pc"], dtype=np.float32)
    valid = np.asarray(inputs["valid"], dtype=np.float32)

    Wp1, bp1 = _fold_bn(*(np.asarray(inputs[k], dtype=np.float32)
                          for k in ("W1", "b1", "gamma1", "beta1", "mean1", "var1")))
    Wp2, bp2 = _fold_bn(*(np.asarray(inputs[k], dtype=np.float32)
                          for k in ("W2", "b2", "gamma2", "beta2", "mean2", "var2")))
    Wp3, bp3 = _fold_bn(*(np.asarray(inputs[k], dtype=np.float32)
                          for k in ("W3", "b3", "gamma3", "beta3", "mean3", "var3")))

    lhsT1 = np.zeros((7, 128), np.float16)
    lhsT1[0:3, 0:64] = Wp1.T
    lhsT1[3:6, 64:128] = Wp1.T
    lhsT1[6, 0:64] = bp1
    lhsT1[6, 64:128] = bp1

    lhsT2 = np.zeros((128, 128), np.float16)
    lhsT2[0:64, 0:64] = Wp2.T
    lhsT2[64:128, 64:128] = Wp2.T

    lhsT3 = np.zeros((128, 128), np.float16)
    lhsT3[0:64, :] = Wp3.T
    lhsT3[64:128, :] = Wp3.T

    biases = np.zeros((128, 2), np.float32)
    biases[:, 0] = np.concatenate([bp2, bp2])
    biases[:, 1] = bp3

    x = gp[0].reshape(3, NCOLS)
    vidx = np.flatnonzero(valid.reshape(NCOLS) > 0.5)
    V = len(vidx)
    Vc = -(-V // N_CORES)
    iters = max(1, -(-Vc // ITER_COLS))
    cap = iters * ITER_COLS

    xv = x[:, vidx].astype(np.float16)

    in_maps = []
    for c in range(N_CORES):
        lo_i = c * Vc
        hi_i = min((c + 1) * Vc, V)
        n = max(0, hi_i - lo_i)
        xa = np.zeros((3, cap), np.float16)
        if n:
            xa[:, :n] = xv[:, lo_i:hi_i]
        xr = xa.reshape(3, iters, 2, M)
        xp = np.ones((7, iters, M), np.float16)
        xp[0:3] = xr[:, :, 0, :]
        xp[3:6] = xr[:, :, 1, :]
        in_maps.append(
            {
                "xp": np.ascontiguousarray(xp.reshape(7, iters * M)),
                "lhsT1": lhsT1,
                "lhsT2": lhsT2,
                "lhsT3": lhsT3,
                "biases": biases,
            }
        )
    return in_maps, vidx, V, Vc, iters


def _gather(results, vidx, V, Vc):
    stream = np.empty((128, V), np.float32)
    for c in range(N_CORES):
        lo_i = c * Vc
        hi_i = min((c + 1) * Vc, V)
        if hi_i <= lo_i:
            break
        stream[:, lo_i:hi_i] = results[c]["out"][:, : hi_i - lo_i]
    full = np.zeros((128, NCOLS), np.float32)
    full[:, vidx] = stream
    return full.reshape(128, NPOINT, KNN)[None]


def run_traced(trace=False, **inputs):
    in_maps, vidx, V, Vc, iters = _prepare(inputs)
    nc = _get_nc(iters)
    res = run_bass_kernel_spmd(nc, in_maps, list(range(N_CORES)), trace=trace)
    return _gather(res.results, vidx, V, Vc), res.exec_time_ns


def kernel(**inputs):
    out, _ = run_traced(trace=False, **inputs)
    return out
